# revision 14
# baseline (speedup 1.0000x reference)
"""2-layer GAT (GATConv x2, 4 heads, concat) over a 100K-node / 1.7M-edge graph
on 8 Trainium2 NeuronCores.

Destination-sharded graph parallelism:
  - Nodes sharded 12500/core; core k owns destinations [12500k, 12500(k+1)).
  - Per layer each core computes h = x_in @ W.T for its own slice; AllGather
    replicates the full feature table (split in two pieces so the second
    half's collective overlaps the producer loop) as bf16 256B rows.
  - The table is addressed by dma_gather int16 indices in 4 chunks:
    half0 = rows 0:4096 of each core (one 32768-row chunk), half1 = rows
    4096:12544 (chunks of 32768/32768/2048).
  - Edge phase per core, per destination block (128 dsts): edges land in
    128-edge groups (partition-major), chunk-major within each 2-block
    super so one batched dma_gather per (super, chunk) fetches all rows.
    Attention pieces:
      s_e   = <h[src_e], att_src>   DVE mul+reduce on the gathered rows
      d_e   = a_dst[dst_e]          broadcast by a PE matmul with a
                                    host-precomputed fp8 0/1 matrix SdT[d,e]
      ex_e  = exp(leaky_relu(s_e + d_e))      (leaky_relu+exp on ACT)
    One PE matmul per group with a host-precomputed fp8 selection matrix
    S01[e,d] accumulates numerators and softmax denominators in PSUM:
      psum[d, 0:128] += sum_e S01[e,d] * ex[e,h] * h[src_e][h,c]
      psum[d,128:132]+= sum_e S01[e,d] * ex[e,h]
    Softmax normalization commutes with the linear aggregation and happens
    per destination after accumulation.
  - Self-loops are ordinary edges; weights replicated; the host does only
    data layout (sharding, sorting, padding, index tables, dtype casts).
"""

import os
import numpy as np
import ml_dtypes

import concourse.bass as bass
import concourse.bacc as bacc
import concourse.tile as tile
import concourse.mybir as mybir
from concourse import library_config
from concourse.bass import IndirectOffsetOnAxis
from concourse.bass_utils import run_bass_kernel_spmd

BF16 = ml_dtypes.bfloat16
FP8 = ml_dtypes.float8_e4m3

N = 100000
EMB_IN = 32
HEADS = 4
C = 32
HID = 128
NEG = 0.2
NCORES = 8
NSH = N // NCORES            # 12500 nodes per shard
NBLK = (NSH + 127) // 128    # 98 dst blocks per shard
NPAD = NBLK * 128            # 12544 padded rows per shard
GSUP = 2                     # blocks per super-batch
NSUP = NBLK // GSUP
PAD_LOC = 300.0              # dst-lane value for padding slots

H0B = 32                     # blocks in table half 0
H0R = H0B * 128              # 4096 rows/core in half 0
H1R = NPAD - H0R             # 8448 rows/core in half 1
CHUNK = 32768
# chunk c>0 covers half-1 rows [(c-1)*32768, ...)
CH_ROWS = [H0R * NCORES, CHUNK, CHUNK, H1R * NCORES - 2 * CHUNK]

_cache = {}

K_SPLITCOLL = os.environ.get("K_SPLITCOLL", "1") == "1"
K_NSUP = int(os.environ.get("K_NSUP", "0")) or None  # limit supers (debug)
K_NOGATHER = os.environ.get("K_NOGATHER", "0") == "1"  # debug: skip gathers


def _table_pos(v):
    """node id -> (chunk, relative row) under the split-table layout."""
    k = v // NSH
    r = v % NSH
    h1 = k * H1R + (r - H0R)
    c = np.where(r < H0R, 0, 1 + h1 // CHUNK)
    rel = np.where(r < H0R, k * H0R + r, h1 % CHUNK)
    return c.astype(np.int64), rel.astype(np.int64)


def _host_layout(x, edge_index):
    """Per-core edge/gather index tables. Pure index manipulation."""
    src = np.concatenate([edge_index[0], np.arange(N, dtype=np.int64)])
    dst = np.concatenate([edge_index[1], np.arange(N, dtype=np.int64)])
    chk, rel = _table_pos(src)

    percore = []
    cnt = np.zeros((NCORES, NBLK, 4), dtype=np.int64)
    for k in range(NCORES):
        lo = k * NSH
        m = (dst >= lo) & (dst < lo + NSH)
        rl, cc = rel[m], chk[m]
        dl = dst[m] - lo
        blk = dl // 128
        order = np.lexsort((cc, blk))
        rl, dl, blk, cc = rl[order], dl[order], blk[order], cc[order]
        for b in range(NBLK):
            bm = blk == b
            cnt[k, b] = np.bincount(cc[bm], minlength=4)
        percore.append((rl, dl, blk, cc))

    # groups per (block, chunk): uniform across cores (SPMD structure)
    gbc = -(-np.max(cnt, axis=0) // 128)          # [NBLK, 4]
    # global group order: super-major, chunk-major, block-minor
    blk_groups = [[] for _ in range(NBLK)]
    grp_blk = []
    sup_specs = []
    gg = 0
    for s in range(NSUP):
        blks = list(range(s * GSUP, (s + 1) * GSUP))
        specs = []
        for c in range(4):
            j0 = gg
            for b in blks:
                for _ in range(int(gbc[b, c])):
                    blk_groups[b].append(gg)
                    grp_blk.append(b)
                    gg += 1
            if gg > j0:
                specs.append((c, j0, gg - j0))
        sup_specs.append(specs)
    gtot = gg

    # slot layout per (block, chunk): cnt real edges then pads
    cores = []
    for k in range(NCORES):
        rl, dl, blk, cc = percore[k]
        idxm = np.zeros((gtot, 128), dtype=np.int16)
        locm = np.full((gtot, 128), PAD_LOC, dtype=np.float32)
        pos = 0
        # edges are sorted by (blk, chunk); walk in the same order
        starts = {}
        for b in range(NBLK):
            for c in range(4):
                n = int(cnt[k, b, c])
                starts[(b, c)] = (pos, n)
                pos += n
        for s in range(NSUP):
            for c, j0, W in sup_specs[s]:
                j = j0
                for b in range(s * GSUP, (s + 1) * GSUP):
                    G = int(gbc[b, c])
                    if G == 0:
                        continue
                    p0, n = starts[(b, c)]
                    vals = np.zeros(G * 128, dtype=np.int16)
                    locs = np.full(G * 128, PAD_LOC, dtype=np.float32)
                    vals[:n] = rl[p0:p0 + n].astype(np.int16)
                    locs[:n] = (dl[p0:p0 + n] - b * 128).astype(np.float32)
                    idxm[j:j + G] = vals.reshape(G, 128)
                    locm[j:j + G] = locs.reshape(G, 128)
                    j += G
        # gidx: idx i of group g -> [16r + i%16, g*8 + i//16]
        gidx16 = idxm.reshape(gtot, 8, 16).transpose(2, 0, 1).reshape(
            16, gtot * 8)
        gidx = np.tile(gidx16, (8, 1))
        loci = locm.astype(np.int64)
        valid = loci < 128
        sdt = np.zeros((128, gtot * 128), dtype=FP8)
        s01 = np.zeros((128, gtot * 128), dtype=FP8)
        ggi, ei = np.nonzero(valid)
        sdt[loci[valid], ggi * 128 + ei] = FP8(1.0)
        s01[ei, ggi * 128 + loci[valid]] = FP8(1.0)
        xsl = np.zeros(NPAD, dtype=np.int32)
        xsl[:NSH] = x[k * NSH:(k + 1) * NSH].astype(np.int32)
        cores.append(dict(gidx=np.ascontiguousarray(gidx),
                          sdt=np.ascontiguousarray(sdt),
                          s01=np.ascontiguousarray(s01),
                          xsh=xsl.reshape(NBLK, 128).T.copy()))
    meta = dict(blk_groups=blk_groups, gtot=gtot, sup_specs=sup_specs,
                grp_blk=grp_blk)
    return meta, cores


def _build(nc, meta):
    dt = mybir.dt
    f32, bf16, i32, i16 = dt.float32, dt.bfloat16, dt.int32, dt.int16
    f8 = dt.float8e4
    gtot = meta["gtot"]
    blk_groups = meta["blk_groups"]
    sup_specs = meta["sup_specs"]
    grp_blk = meta["grp_blk"]
    AF = mybir.ActivationFunctionType

    emb_t = nc.dram_tensor("emb", [N, EMB_IN], f32, kind="ExternalInput")
    w1_t = nc.dram_tensor("w1", [HID, EMB_IN], f32, kind="ExternalInput")
    w1t_t = nc.dram_tensor("w1t", [EMB_IN, HID], f32, kind="ExternalInput")
    w2_t = nc.dram_tensor("w2", [HID, HID], f32, kind="ExternalInput")
    w2t_t = nc.dram_tensor("w2t", [HID, HID], f32, kind="ExternalInput")
    a1d_t = nc.dram_tensor("a1d", [HID, HEADS], f32, kind="ExternalInput")
    a2d_t = nc.dram_tensor("a2d", [HID, HEADS], f32, kind="ExternalInput")
    at1_t = nc.dram_tensor("attrep1", [128, HID], bf16, kind="ExternalInput")
    at2_t = nc.dram_tensor("attrep2", [128, HID], bf16, kind="ExternalInput")
    b1r_t = nc.dram_tensor("b1r", [128, HID], f32, kind="ExternalInput")
    b2r_t = nc.dram_tensor("b2r", [128, HID], f32, kind="ExternalInput")
    idf_t = nc.dram_tensor("identf", [128, 128], f32, kind="ExternalInput")
    xsh_t = nc.dram_tensor("xsh", [128, NBLK], i32, kind="ExternalInput")
    gidx_t = nc.dram_tensor("gidx", [128, gtot * 8], i16, kind="ExternalInput")
    sdt_t = nc.dram_tensor("sdt", [128, gtot * 128], f8, kind="ExternalInput")
    s01_t = nc.dram_tensor("s01", [128, gtot * 128], f8, kind="ExternalInput")
    out_t = nc.dram_tensor("out2", [NPAD, HID], f32, kind="ExternalOutput")

    # producer slices (split so the first collective can fire early) and the
    # gathered tables (half0 = one 32768-row chunk, half1 = 3 chunks)
    gslA = [nc.dram_tensor(f"gslA{l}", [H0R, HID], bf16, kind="Internal")
            for l in (1, 2)]
    gslB = [nc.dram_tensor(f"gslB{l}", [H1R, HID], bf16, kind="Internal")
            for l in (1, 2)]
    gf0 = [nc.dram_tensor(f"gf0_{l}", [H0R * NCORES, HID], bf16,
                          kind="Internal", addr_space="Shared") for l in (1, 2)]
    gf1 = [nc.dram_tensor(f"gf1_{l}", [H1R * NCORES, HID], bf16,
                          kind="Internal", addr_space="Shared") for l in (1, 2)]

    def bmid(ap, w):
        """[128, X] -> [128, w, X] broadcast over a middle dim."""
        return bass.AP(ap.tensor, ap.offset, [list(ap.ap[0]), [0, w],
                                              list(ap.ap[1])])

    def allgather(src, dstt):
        nc.gpsimd.collective_compute(
            "AllGather", mybir.AluOpType.bypass,
            replica_groups=[list(range(NCORES))],
            ins=[src.ap()], outs=[dstt.ap()])

    with tile.TileContext(nc) as tc:
        with tc.tile_pool(name="const", bufs=1) as cpool, \
             tc.tile_pool(name="work", bufs=2) as wpool, \
             tc.tile_pool(name="psum", bufs=2, space="PSUM") as ppool:

            def cload(t, shape, dtyp):
                s = cpool.tile(shape, dtyp, tag=t.name)
                nc.sync.dma_start(s[:], t[:])
                return s

            w1_sb = cload(w1_t, [HID, EMB_IN], f32)
            w1t_sb = cload(w1t_t, [EMB_IN, HID], f32)
            w2_sb = cload(w2_t, [HID, HID], f32)
            a1d_sb = cload(a1d_t, [HID, HEADS], f32)
            a2d_sb = cload(a2d_t, [HID, HEADS], f32)
            at1_sb = cload(at1_t, [128, HID], bf16)
            at2_sb = cload(at2_t, [128, HID], bf16)
            at_sb = [at1_sb, at2_sb]
            b1r_sb = cload(b1r_t, [128, HID], f32)
            b2r_sb = cload(b2r_t, [128, HID], f32)
            idf_sb = cload(idf_t, [128, 128], f32)
            xsh_sb = cload(xsh_t, [128, NBLK], i32)
            gidx_sb = cload(gidx_t, [128, gtot * 8], i16)

            w2t_f = cpool.tile([HID, HID], f32, tag="w2tf")
            nc.sync.dma_start(w2t_f[:], w2t_t[:])
            w2t_bf = cpool.tile([HID, HID], bf16, tag="w2tbf")
            nc.vector.tensor_copy(w2t_bf[:], w2t_f[:])

            # M1d = W1^T A1dst [32, 4];  M2d = W2^T A2dst [128, 4]
            m1_ps = ppool.tile([EMB_IN, HEADS], f32, tag="tp")
            nc.tensor.matmul(out=m1_ps[:], lhsT=w1_sb[:], rhs=a1d_sb[:],
                             start=True, stop=True)
            m1_sb = cpool.tile([EMB_IN, HEADS], f32, tag="m1s")
            nc.vector.tensor_copy(m1_sb[:], m1_ps[:])
            m2_ps = ppool.tile([HID, HEADS], f32, tag="tp")
            nc.tensor.matmul(out=m2_ps[:], lhsT=w2_sb[:], rhs=a2d_sb[:],
                             start=True, stop=True)
            m2_bf = cpool.tile([HID, HEADS], bf16, tag="m2b")
            nc.vector.tensor_copy(m2_bf[:], m2_ps[:])

            # a_dst for own blocks, per layer: [128, NBLK, 4]
            adsb1 = cpool.tile([128, NBLK, HEADS], bf16, tag="adsb1")
            adsb2 = cpool.tile([128, NBLK, HEADS], bf16, tag="adsb2")
            adsb = [adsb1, adsb2]

            # ---- phase A: layer-1 h slice for own nodes ------------------
            for g in range(NBLK):
                embx = wpool.tile([128, EMB_IN], f32, tag="embx")
                nc.gpsimd.indirect_dma_start(
                    out=embx[:], out_offset=None, in_=emb_t[:],
                    in_offset=IndirectOffsetOnAxis(ap=xsh_sb[:, g:g + 1], axis=0))
                tp = ppool.tile([EMB_IN, 128], f32, tag="tp")
                nc.tensor.transpose(tp[:], embx[:], idf_sb[:])
                exT = wpool.tile([EMB_IN, 128], f32, tag="exT")
                nc.vector.tensor_copy(exT[:], tp[:])
                hp = ppool.tile([128, HID + HEADS], f32, tag="hp")
                nc.tensor.matmul(out=hp[:, 0:HID], lhsT=exT[:], rhs=w1t_sb[:],
                                 start=True, stop=True)
                nc.tensor.matmul(out=hp[:, HID:HID + HEADS], lhsT=exT[:],
                                 rhs=m1_sb[:], start=True, stop=True)
                sl = wpool.tile([128, HID], bf16, tag="slice")
                nc.vector.tensor_copy(sl[:], hp[:, 0:HID])
                nc.vector.tensor_copy(adsb[0][:, g, :], hp[:, HID:HID + HEADS])
                if g < H0B:
                    nc.sync.dma_start(gslA[0][g * 128:(g + 1) * 128, :], sl[:])
                else:
                    gb = g - H0B
                    nc.sync.dma_start(gslB[0][gb * 128:(gb + 1) * 128, :],
                                      sl[:])
                if g == H0B - 1 and K_SPLITCOLL:
                    allgather(gslA[0], gf0[0])
            if not K_SPLITCOLL:
                allgather(gslA[0], gf0[0])
            allgather(gslB[0], gf1[0])

            # ---- edge phase ---------------------------------------------
            for layer in (0, 1):
                cviews = [gf0[layer][:],
                          gf1[layer][0:CHUNK, :],
                          gf1[layer][CHUNK:2 * CHUNK, :],
                          gf1[layer][2 * CHUNK:, :]]
                for s in range(K_NSUP or NSUP):
                    gg0 = sup_specs[s][0][1]
                    ggE = sup_specs[s][-1][1] + sup_specs[s][-1][2]
                    Ws = ggE - gg0
                    gath = wpool.tile([128, Ws, HID], bf16, tag="gath")
                    if not K_NOGATHER:
                        for c, j0, W in sup_specs[s]:
                            nc.gpsimd.dma_gather(
                                gath[:, j0 - gg0:j0 - gg0 + W, :], cviews[c],
                                gidx_sb[:, j0 * 8:(j0 + W) * 8],
                                W * 128, W * 128, HID,
                                single_packet=False)
                    sdt_sb = wpool.tile([128, Ws, 128], f8, tag="sdt")
                    nc.sync.dma_start(
                        sdt_sb[:],
                        sdt_t[:, gg0 * 128:ggE * 128]
                        .rearrange("p (w e) -> p w e", w=Ws))
                    s01_sb = wpool.tile([128, Ws, 128], f8, tag="s01")
                    nc.sync.dma_start(
                        s01_sb[:],
                        s01_t[:, gg0 * 128:ggE * 128]
                        .rearrange("p (w e) -> p w e", w=Ws))
                    # d_e = a_dst[dst_e] via fp8 selection matmul
                    dps = ppool.tile([128, Ws, HEADS], f32, tag="dp")
                    for j in range(Ws):
                        nc.tensor.matmul(out=dps[:, j, :],
                                         lhsT=sdt_sb[:, j, :],
                                         rhs=adsb[layer][:, grp_blk[gg0 + j], :],
                                         start=True, stop=True)
                    # s_e = <h_src, att_src>
                    hm = wpool.tile([128, Ws, HID], bf16, tag="hm")
                    nc.vector.tensor_mul(hm[:], gath[:],
                                         bmid(at_sb[layer][:], Ws))
                    s_sb = wpool.tile([128, Ws, HEADS], f32, tag="s")
                    nc.vector.tensor_reduce(
                        s_sb[:], hm[:].rearrange("p w (h c) -> p w h c", h=HEADS),
                        axis=mybir.AxisListType.X, op=mybir.AluOpType.add)
                    # ex = exp(leaky_relu(s + d))
                    z = wpool.tile([128, Ws, HEADS], f32, tag="z")
                    nc.vector.tensor_add(z[:], s_sb[:], dps[:])
                    zm = wpool.tile([128, Ws, HEADS], f32, tag="zm")
                    nc.vector.tensor_scalar_mul(zm[:], z[:], NEG)
                    nc.vector.tensor_max(z[:], z[:], zm[:])
                    ex = wpool.tile([128, Ws, HEADS], bf16, tag="ex")
                    nc.scalar.activation(ex[:], z[:], AF.Exp)
                    # rhs = [h * ex | ex]
                    rhs = wpool.tile([128, Ws, HID + HEADS], bf16, tag="rhs")
                    nc.vector.tensor_mul(
                        rhs[:, :, 0:HID].rearrange("p w (h c) -> p w h c",
                                                   h=HEADS),
                        gath[:].rearrange("p w (h c) -> p w h c", h=HEADS),
                        ex[:].to_broadcast([128, Ws, HEADS, C]))
                    nc.vector.tensor_copy(rhs[:, :, HID:HID + HEADS], ex[:])
                    # aggregate per block
                    for b in range(s * GSUP, (s + 1) * GSUP):
                        ggs = blk_groups[b]
                        agg = ppool.tile([128, HID + HEADS], f32, tag="agg")
                        for i, gg in enumerate(ggs):
                            j = gg - gg0
                            nc.tensor.matmul(out=agg[:], lhsT=s01_sb[:, j, :],
                                             rhs=rhs[:, j, :],
                                             start=(i == 0),
                                             stop=(i == len(ggs) - 1))
                        den = wpool.tile([128, HEADS], f32, tag="den")
                        nc.vector.tensor_scalar_add(
                            den[:], agg[:, HID:HID + HEADS], 1e-16)
                        rec = wpool.tile([128, HEADS], f32, tag="rec")
                        nc.vector.reciprocal(rec[:], den[:])
                        recr = wpool.tile([128, HID], f32, tag="recr")
                        nc.vector.tensor_copy(
                            recr[:].rearrange("p (h c) -> p h c", h=HEADS),
                            rec[:].to_broadcast([128, HEADS, C]))
                        normed = wpool.tile([128, HID], f32, tag="normed")
                        nc.vector.tensor_mul(normed[:], agg[:, 0:HID], recr[:])
                        if layer == 0:
                            nc.vector.tensor_add(normed[:], normed[:],
                                                 b1r_sb[:])
                            relu = wpool.tile([128, HID], f32, tag="relu")
                            nc.vector.tensor_scalar_max(relu[:], normed[:],
                                                        0.0)
                            tp2 = ppool.tile([128, 128], f32, tag="tp")
                            nc.tensor.transpose(tp2[:], relu[:], idf_sb[:])
                            rT = wpool.tile([128, 128], bf16, tag="rT")
                            nc.vector.tensor_copy(rT[:], tp2[:])
                            hp2 = ppool.tile([128, HID + HEADS], f32, tag="hp")
                            nc.tensor.matmul(out=hp2[:, 0:HID], lhsT=rT[:],
                                             rhs=w2t_bf[:], start=True,
                                             stop=True)
                            nc.tensor.matmul(out=hp2[:, HID:HID + HEADS],
                                             lhsT=rT[:], rhs=m2_bf[:],
                                             start=True, stop=True)
                            sl2 = wpool.tile([128, HID], bf16, tag="slice")
                            nc.vector.tensor_copy(sl2[:], hp2[:, 0:HID])
                            nc.vector.tensor_copy(adsb[1][:, b, :],
                                                  hp2[:, HID:HID + HEADS])
                            if b < H0B:
                                nc.sync.dma_start(
                                    gslA[1][b * 128:(b + 1) * 128, :], sl2[:])
                            else:
                                bb = b - H0B
                                nc.sync.dma_start(
                                    gslB[1][bb * 128:(bb + 1) * 128, :],
                                    sl2[:])
                        else:
                            outb = wpool.tile([128, HID], f32, tag="outb")
                            nc.vector.tensor_add(outb[:], normed[:], b2r_sb[:])
                            nc.sync.dma_start(
                                out_t[b * 128:(b + 1) * 128, :], outb[:])
                    if (layer == 0 and K_SPLITCOLL
                            and s == H0B // GSUP - 1):
                        allgather(gslA[1], gf0[1])
                if layer == 0:
                    if not K_SPLITCOLL:
                        allgather(gslA[1], gf0[1])
                    allgather(gslB[1], gf1[1])
    nc.finalize()
    return nc


def kernel(**inputs):
    x = np.asarray(inputs["x"])
    edge_index = np.asarray(inputs["edge_index"])
    emb = np.asarray(inputs["emb"], dtype=np.float32)
    W1 = np.asarray(inputs["W1"], dtype=np.float32)
    W2 = np.asarray(inputs["W2"], dtype=np.float32)
    as1 = np.asarray(inputs["att_src1"], dtype=np.float32)
    ad1 = np.asarray(inputs["att_dst1"], dtype=np.float32)
    as2 = np.asarray(inputs["att_src2"], dtype=np.float32)
    ad2 = np.asarray(inputs["att_dst2"], dtype=np.float32)
    b1 = np.asarray(inputs["b1"], dtype=np.float32)
    b2 = np.asarray(inputs["b2"], dtype=np.float32)

    key = (edge_index.tobytes(), x.tobytes())
    if _cache.get("key") != key:
        meta, cores = _host_layout(x, edge_index)
        nc = _build(bacc.Bacc("TRN2", target_bir_lowering=False, debug=False,
                              enable_asserts=False, num_devices=NCORES), meta)
        _cache.update(key=key, nc=nc, cores=cores)
    nc, cores = _cache["nc"], _cache["cores"]

    common = dict(
        emb=emb, w1=W1, w1t=np.ascontiguousarray(W1.T),
        w2=W2, w2t=np.ascontiguousarray(W2.T),
        a1d=_amat_d(ad1),
        a2d=_amat_d(ad2),
        attrep1=np.ascontiguousarray(
            np.broadcast_to(as1.reshape(-1), (128, HID))).astype(BF16),
        attrep2=np.ascontiguousarray(
            np.broadcast_to(as2.reshape(-1), (128, HID))).astype(BF16),
        b1r=np.ascontiguousarray(np.broadcast_to(b1, (128, HID))),
        b2r=np.ascontiguousarray(np.broadcast_to(b2, (128, HID))),
        identf=np.eye(128, dtype=np.float32),
    )
    in_maps = [dict(common, **cores[k]) for k in range(NCORES)]

    res = run_bass_kernel_spmd(nc, in_maps, core_ids=list(range(NCORES)),
                               tmpdir=os.environ.get("BASS_TRACE_DIR"))
    global LAST_EXEC_NS, LAST_TRACE
    LAST_EXEC_NS = res.exec_time_ns
    LAST_TRACE = (res.instructions_and_trace[1]
                  if res.instructions_and_trace else None)
    out = np.concatenate([res.results[k]["out2"][:NSH] for k in range(NCORES)],
                         axis=0)
    return out.astype(np.float32)


LAST_EXEC_NS = None
LAST_TRACE = None


def _amat_d(adst):
    A = np.zeros((HID, HEADS), dtype=np.float32)
    for h in range(HEADS):
        A[h * C:(h + 1) * C, h] = adst[h]
    return A


if __name__ == "__main__":
    import reference
    inputs = {k: np.asarray(v) for k, v in reference.setup_inputs().items()}
    got = kernel(**inputs)
    print("out shape", got.shape, got.dtype)


# revision 17
# speedup vs baseline: 1.3828x; 1.3828x over previous
"""2-layer GAT (GATConv x2, 4 heads, concat) over a 100K-node / 1.7M-edge graph
on 8 Trainium2 NeuronCores.

Destination-sharded graph parallelism:
  - Nodes sharded 12500/core; core k owns destinations [12500k, 12500(k+1)).
  - Per layer each core computes h = x_in @ W.T for its own slice; AllGather
    replicates the full feature table (split in two pieces so the second
    half's collective overlaps the producer loop) as bf16 256B rows.
  - The table is addressed by dma_gather int16 indices in 4 chunks:
    half0 = rows 0:4096 of each core (one 32768-row chunk), half1 = rows
    4096:12544 (chunks of 32768/32768/2048).
  - Edge phase per core, per destination block (128 dsts): edges land in
    128-edge groups (partition-major), chunk-major within each 2-block
    super so one batched dma_gather per (super, chunk) fetches all rows.
    Attention pieces:
      s_e   = <h[src_e], att_src>   DVE mul+reduce on the gathered rows
      d_e   = a_dst[dst_e]          broadcast by a PE matmul with a
                                    host-precomputed fp8 0/1 matrix SdT[d,e]
      ex_e  = exp(leaky_relu(s_e + d_e))      (leaky_relu+exp on ACT)
    One PE matmul per group with a host-precomputed fp8 selection matrix
    S01[e,d] accumulates numerators and softmax denominators in PSUM:
      psum[d, 0:128] += sum_e S01[e,d] * ex[e,h] * h[src_e][h,c]
      psum[d,128:132]+= sum_e S01[e,d] * ex[e,h]
    Softmax normalization commutes with the linear aggregation and happens
    per destination after accumulation.
  - Self-loops are ordinary edges; weights replicated; the host does only
    data layout (sharding, sorting, padding, index tables, dtype casts).
"""

import os
import numpy as np
import ml_dtypes

import concourse.bass as bass
import concourse.bacc as bacc
import concourse.tile as tile
import concourse.mybir as mybir
from concourse import library_config
from concourse.bass import IndirectOffsetOnAxis
from concourse.bass_utils import run_bass_kernel_spmd

BF16 = ml_dtypes.bfloat16
FP8 = ml_dtypes.float8_e4m3

N = 100000
EMB_IN = 32
HEADS = 4
C = 32
HID = 128
NEG = 0.2
NCORES = 8
NSH = N // NCORES            # 12500 nodes per shard
NBLK = (NSH + 127) // 128    # 98 dst blocks per shard
NPAD = NBLK * 128            # 12544 padded rows per shard
GSUP = 2                     # blocks per super-batch
NSUP = NBLK // GSUP
PAD_LOC = 300.0              # dst-lane value for padding slots

H0B = 32                     # blocks in table half 0
H0R = H0B * 128              # 4096 rows/core in half 0
H1R = NPAD - H0R             # 8448 rows/core in half 1
CHUNK = 32768
# chunk c>0 covers half-1 rows [(c-1)*32768, ...)
CH_ROWS = [H0R * NCORES, CHUNK, CHUNK, H1R * NCORES - 2 * CHUNK]

_cache = {}

K_SPLITCOLL = os.environ.get("K_SPLITCOLL", "1") == "1"
K_NSUP = int(os.environ.get("K_NSUP", "0")) or None  # limit supers (debug)
K_NOGATHER = os.environ.get("K_NOGATHER", "0") == "1"  # debug: skip gathers


def _table_pos(v):
    """node id -> (chunk, relative row) under the split-table layout."""
    k = v // NSH
    r = v % NSH
    h1 = k * H1R + (r - H0R)
    c = np.where(r < H0R, 0, 1 + h1 // CHUNK)
    rel = np.where(r < H0R, k * H0R + r, h1 % CHUNK)
    return c.astype(np.int64), rel.astype(np.int64)


def _host_layout(x, edge_index):
    """Per-core edge/gather index tables. Pure index manipulation."""
    src = np.concatenate([edge_index[0], np.arange(N, dtype=np.int64)])
    dst = np.concatenate([edge_index[1], np.arange(N, dtype=np.int64)])
    chk, rel = _table_pos(src)

    percore = []
    cnt = np.zeros((NCORES, NBLK, 4), dtype=np.int64)
    for k in range(NCORES):
        lo = k * NSH
        m = (dst >= lo) & (dst < lo + NSH)
        rl, cc = rel[m], chk[m]
        dl = dst[m] - lo
        blk = dl // 128
        order = np.lexsort((cc, blk))
        rl, dl, blk, cc = rl[order], dl[order], blk[order], cc[order]
        for b in range(NBLK):
            bm = blk == b
            cnt[k, b] = np.bincount(cc[bm], minlength=4)
        percore.append((rl, dl, blk, cc))

    # groups per (block, chunk): uniform across cores (SPMD structure)
    gbc = -(-np.max(cnt, axis=0) // 128)          # [NBLK, 4]
    # global group order: super-major, chunk-major, block-minor
    blk_groups = [[] for _ in range(NBLK)]
    grp_blk = []
    sup_specs = []
    gg = 0
    for s in range(NSUP):
        blks = list(range(s * GSUP, (s + 1) * GSUP))
        specs = []
        for c in range(4):
            j0 = gg
            for b in blks:
                for _ in range(int(gbc[b, c])):
                    blk_groups[b].append(gg)
                    grp_blk.append(b)
                    gg += 1
            if gg > j0:
                specs.append((c, j0, gg - j0))
        sup_specs.append(specs)
    gtot = gg

    # slot layout per (block, chunk): cnt real edges then pads
    cores = []
    for k in range(NCORES):
        rl, dl, blk, cc = percore[k]
        idxm = np.zeros((gtot, 128), dtype=np.int16)
        locm = np.full((gtot, 128), PAD_LOC, dtype=np.float32)
        pos = 0
        # edges are sorted by (blk, chunk); walk in the same order
        starts = {}
        for b in range(NBLK):
            for c in range(4):
                n = int(cnt[k, b, c])
                starts[(b, c)] = (pos, n)
                pos += n
        for s in range(NSUP):
            for c, j0, W in sup_specs[s]:
                j = j0
                for b in range(s * GSUP, (s + 1) * GSUP):
                    G = int(gbc[b, c])
                    if G == 0:
                        continue
                    p0, n = starts[(b, c)]
                    vals = np.zeros(G * 128, dtype=np.int16)
                    locs = np.full(G * 128, PAD_LOC, dtype=np.float32)
                    vals[:n] = rl[p0:p0 + n].astype(np.int16)
                    locs[:n] = (dl[p0:p0 + n] - b * 128).astype(np.float32)
                    idxm[j:j + G] = vals.reshape(G, 128)
                    locm[j:j + G] = locs.reshape(G, 128)
                    j += G
        # gidx: idx i of group g -> [16r + i%16, g*8 + i//16]
        gidx16 = idxm.reshape(gtot, 8, 16).transpose(2, 0, 1).reshape(
            16, gtot * 8)
        gidx = np.tile(gidx16, (8, 1))
        loci = locm.astype(np.int64)
        valid = loci < 128
        sdt = np.zeros((128, gtot * 128), dtype=FP8)
        s01 = np.zeros((128, gtot * 128), dtype=FP8)
        ggi, ei = np.nonzero(valid)
        sdt[loci[valid], ggi * 128 + ei] = FP8(1.0)
        s01[ei, ggi * 128 + loci[valid]] = FP8(1.0)
        xsl = np.zeros(NPAD, dtype=np.int32)
        xsl[:NSH] = x[k * NSH:(k + 1) * NSH].astype(np.int32)
        cores.append(dict(gidx=np.ascontiguousarray(gidx),
                          sdt=np.ascontiguousarray(sdt),
                          s01=np.ascontiguousarray(s01),
                          xsh=xsl.reshape(NBLK, 128).T.copy()))
    meta = dict(blk_groups=blk_groups, gtot=gtot, sup_specs=sup_specs,
                grp_blk=grp_blk)
    return meta, cores


def _build(nc, meta):
    dt = mybir.dt
    f32, bf16, i32, i16 = dt.float32, dt.bfloat16, dt.int32, dt.int16
    f8 = dt.float8e4
    gtot = meta["gtot"]
    blk_groups = meta["blk_groups"]
    sup_specs = meta["sup_specs"]
    grp_blk = meta["grp_blk"]
    AF = mybir.ActivationFunctionType

    emb_t = nc.dram_tensor("emb", [N, EMB_IN], f32, kind="ExternalInput")
    w1_t = nc.dram_tensor("w1", [HID, EMB_IN], f32, kind="ExternalInput")
    w1t_t = nc.dram_tensor("w1t", [EMB_IN, HID], f32, kind="ExternalInput")
    w2_t = nc.dram_tensor("w2", [HID, HID], f32, kind="ExternalInput")
    w2t_t = nc.dram_tensor("w2t", [HID, HID], f32, kind="ExternalInput")
    a1d_t = nc.dram_tensor("a1d", [HID, HEADS], f32, kind="ExternalInput")
    a2d_t = nc.dram_tensor("a2d", [HID, HEADS], f32, kind="ExternalInput")
    at1_t = nc.dram_tensor("attrep1", [128, HID], bf16, kind="ExternalInput")
    at2_t = nc.dram_tensor("attrep2", [128, HID], bf16, kind="ExternalInput")
    b1r_t = nc.dram_tensor("b1r", [128, HID], f32, kind="ExternalInput")
    b2r_t = nc.dram_tensor("b2r", [128, HID], f32, kind="ExternalInput")
    idf_t = nc.dram_tensor("identf", [128, 128], f32, kind="ExternalInput")
    xsh_t = nc.dram_tensor("xsh", [128, NBLK], i32, kind="ExternalInput")
    gidx_t = nc.dram_tensor("gidx", [128, gtot * 8], i16, kind="ExternalInput")
    sdt_t = nc.dram_tensor("sdt", [128, gtot * 128], f8, kind="ExternalInput")
    s01_t = nc.dram_tensor("s01", [128, gtot * 128], f8, kind="ExternalInput")
    out_t = nc.dram_tensor("out2", [NPAD, HID], f32, kind="ExternalOutput")

    # producer slices (split so the first collective can fire early) and the
    # gathered tables (half0 = one 32768-row chunk, half1 = 3 chunks)
    gslA = [nc.dram_tensor(f"gslA{l}", [H0R, HID], bf16, kind="Internal")
            for l in (1, 2)]
    gslB = [nc.dram_tensor(f"gslB{l}", [H1R, HID], bf16, kind="Internal")
            for l in (1, 2)]
    gf0 = [nc.dram_tensor(f"gf0_{l}", [H0R * NCORES, HID], bf16,
                          kind="Internal", addr_space="Shared") for l in (1, 2)]
    gf1 = [nc.dram_tensor(f"gf1_{l}", [H1R * NCORES, HID], bf16,
                          kind="Internal", addr_space="Shared") for l in (1, 2)]

    def bmid(ap, w):
        """[128, X] -> [128, w, X] broadcast over a middle dim."""
        return bass.AP(ap.tensor, ap.offset, [list(ap.ap[0]), [0, w],
                                              list(ap.ap[1])])

    def allgather(src, dstt):
        nc.gpsimd.collective_compute(
            "AllGather", mybir.AluOpType.bypass,
            replica_groups=[list(range(NCORES))],
            ins=[src.ap()], outs=[dstt.ap()])

    with tile.TileContext(nc) as tc:
        with tc.tile_pool(name="const", bufs=1) as cpool, \
             tc.tile_pool(name="work", bufs=2) as wpool, \
             tc.tile_pool(name="psum", bufs=2, space="PSUM") as ppool:

            def cload(t, shape, dtyp):
                s = cpool.tile(shape, dtyp, tag=t.name)
                nc.sync.dma_start(s[:], t[:])
                return s

            w1_sb = cload(w1_t, [HID, EMB_IN], f32)
            w1t_sb = cload(w1t_t, [EMB_IN, HID], f32)
            w2_sb = cload(w2_t, [HID, HID], f32)
            a1d_sb = cload(a1d_t, [HID, HEADS], f32)
            a2d_sb = cload(a2d_t, [HID, HEADS], f32)
            at1_sb = cload(at1_t, [128, HID], bf16)
            at2_sb = cload(at2_t, [128, HID], bf16)
            at_sb = [at1_sb, at2_sb]
            b1r_sb = cload(b1r_t, [128, HID], f32)
            b2r_sb = cload(b2r_t, [128, HID], f32)
            idf_sb = cload(idf_t, [128, 128], f32)
            xsh_sb = cload(xsh_t, [128, NBLK], i32)
            gidx_sb = cload(gidx_t, [128, gtot * 8], i16)

            w2t_f = cpool.tile([HID, HID], f32, tag="w2tf")
            nc.sync.dma_start(w2t_f[:], w2t_t[:])
            w2t_bf = cpool.tile([HID, HID], bf16, tag="w2tbf")
            nc.vector.tensor_copy(w2t_bf[:], w2t_f[:])

            # M1d = W1^T A1dst [32, 4];  M2d = W2^T A2dst [128, 4]
            m1_ps = ppool.tile([EMB_IN, HEADS], f32, tag="tp")
            nc.tensor.matmul(out=m1_ps[:], lhsT=w1_sb[:], rhs=a1d_sb[:],
                             start=True, stop=True)
            m1_sb = cpool.tile([EMB_IN, HEADS], f32, tag="m1s")
            nc.vector.tensor_copy(m1_sb[:], m1_ps[:])
            m2_ps = ppool.tile([HID, HEADS], f32, tag="tp")
            nc.tensor.matmul(out=m2_ps[:], lhsT=w2_sb[:], rhs=a2d_sb[:],
                             start=True, stop=True)
            m2_bf = cpool.tile([HID, HEADS], bf16, tag="m2b")
            nc.vector.tensor_copy(m2_bf[:], m2_ps[:])

            # a_dst for own blocks, per layer: [128, NBLK, 4]
            adsb1 = cpool.tile([128, NBLK, HEADS], bf16, tag="adsb1")
            adsb2 = cpool.tile([128, NBLK, HEADS], bf16, tag="adsb2")
            adsb = [adsb1, adsb2]

            # ---- phase A: layer-1 h slice for own nodes ------------------
            for g in range(NBLK):
                embx = wpool.tile([128, EMB_IN], f32, tag="embx")
                nc.gpsimd.indirect_dma_start(
                    out=embx[:], out_offset=None, in_=emb_t[:],
                    in_offset=IndirectOffsetOnAxis(ap=xsh_sb[:, g:g + 1], axis=0))
                tp = ppool.tile([EMB_IN, 128], f32, tag="tp")
                nc.tensor.transpose(tp[:], embx[:], idf_sb[:])
                exT = wpool.tile([EMB_IN, 128], f32, tag="exT")
                nc.vector.tensor_copy(exT[:], tp[:])
                hp = ppool.tile([128, HID + HEADS], f32, tag="hp")
                nc.tensor.matmul(out=hp[:, 0:HID], lhsT=exT[:], rhs=w1t_sb[:],
                                 start=True, stop=True)
                nc.tensor.matmul(out=hp[:, HID:HID + HEADS], lhsT=exT[:],
                                 rhs=m1_sb[:], start=True, stop=True)
                sl = wpool.tile([128, HID], bf16, tag="slice")
                nc.vector.tensor_copy(sl[:], hp[:, 0:HID])
                nc.vector.tensor_copy(adsb[0][:, g, :], hp[:, HID:HID + HEADS])
                if g < H0B:
                    nc.sync.dma_start(gslA[0][g * 128:(g + 1) * 128, :], sl[:])
                else:
                    gb = g - H0B
                    nc.sync.dma_start(gslB[0][gb * 128:(gb + 1) * 128, :],
                                      sl[:])
                if g == H0B - 1 and K_SPLITCOLL:
                    allgather(gslA[0], gf0[0])
            if not K_SPLITCOLL:
                allgather(gslA[0], gf0[0])
            allgather(gslB[0], gf1[0])

            # ---- edge phase ---------------------------------------------
            qload = np.zeros(4, dtype=np.int64)
            for layer in (0, 1):
                cviews = [gf0[layer][:],
                          gf1[layer][0:CHUNK, :],
                          gf1[layer][CHUNK:2 * CHUNK, :],
                          gf1[layer][2 * CHUNK:, :]]
                for s in range(K_NSUP or NSUP):
                    gg0 = sup_specs[s][0][1]
                    ggE = sup_specs[s][-1][1] + sup_specs[s][-1][2]
                    Ws = ggE - gg0
                    gath = wpool.tile([128, Ws, HID], bf16, tag="gath")
                    if not K_NOGATHER:
                        for c, j0, W in sup_specs[s]:
                            q = int(np.argmin(qload))
                            qload[q] += W
                            nc.gpsimd.dma_gather(
                                gath[:, j0 - gg0:j0 - gg0 + W, :], cviews[c],
                                gidx_sb[:, j0 * 8:(j0 + W) * 8],
                                W * 128, W * 128, HID,
                                single_packet=False, queue_num=q)
                    sdt_sb = wpool.tile([128, Ws, 128], f8, tag="sdt")
                    nc.sync.dma_start(
                        sdt_sb[:],
                        sdt_t[:, gg0 * 128:ggE * 128]
                        .rearrange("p (w e) -> p w e", w=Ws))
                    s01_sb = wpool.tile([128, Ws, 128], f8, tag="s01")
                    nc.sync.dma_start(
                        s01_sb[:],
                        s01_t[:, gg0 * 128:ggE * 128]
                        .rearrange("p (w e) -> p w e", w=Ws))
                    # d_e = a_dst[dst_e] via fp8 selection matmul
                    dps = ppool.tile([128, Ws, HEADS], f32, tag="dp")
                    for j in range(Ws):
                        nc.tensor.matmul(out=dps[:, j, :],
                                         lhsT=sdt_sb[:, j, :],
                                         rhs=adsb[layer][:, grp_blk[gg0 + j], :],
                                         start=True, stop=True)
                    # s_e = <h_src, att_src>
                    hm = wpool.tile([128, Ws, HID], bf16, tag="hm")
                    nc.vector.tensor_mul(hm[:], gath[:],
                                         bmid(at_sb[layer][:], Ws))
                    s_sb = wpool.tile([128, Ws, HEADS], f32, tag="s")
                    nc.vector.tensor_reduce(
                        s_sb[:], hm[:].rearrange("p w (h c) -> p w h c", h=HEADS),
                        axis=mybir.AxisListType.X, op=mybir.AluOpType.add)
                    # ex = exp(leaky_relu(s + d))
                    z = wpool.tile([128, Ws, HEADS], f32, tag="z")
                    nc.vector.tensor_add(z[:], s_sb[:], dps[:])
                    zm = wpool.tile([128, Ws, HEADS], f32, tag="zm")
                    nc.vector.tensor_scalar_mul(zm[:], z[:], NEG)
                    nc.vector.tensor_max(z[:], z[:], zm[:])
                    ex = wpool.tile([128, Ws, HEADS], bf16, tag="ex")
                    nc.scalar.activation(ex[:], z[:], AF.Exp)
                    # rhs = [h * ex | ex]
                    rhs = wpool.tile([128, Ws, HID + HEADS], bf16, tag="rhs")
                    nc.vector.tensor_mul(
                        rhs[:, :, 0:HID].rearrange("p w (h c) -> p w h c",
                                                   h=HEADS),
                        gath[:].rearrange("p w (h c) -> p w h c", h=HEADS),
                        ex[:].to_broadcast([128, Ws, HEADS, C]))
                    nc.vector.tensor_copy(rhs[:, :, HID:HID + HEADS], ex[:])
                    # aggregate per block
                    for b in range(s * GSUP, (s + 1) * GSUP):
                        ggs = blk_groups[b]
                        agg = ppool.tile([128, HID + HEADS], f32, tag="agg")
                        for i, gg in enumerate(ggs):
                            j = gg - gg0
                            nc.tensor.matmul(out=agg[:], lhsT=s01_sb[:, j, :],
                                             rhs=rhs[:, j, :],
                                             start=(i == 0),
                                             stop=(i == len(ggs) - 1))
                        den = wpool.tile([128, HEADS], f32, tag="den")
                        nc.vector.tensor_scalar_add(
                            den[:], agg[:, HID:HID + HEADS], 1e-16)
                        rec = wpool.tile([128, HEADS], f32, tag="rec")
                        nc.vector.reciprocal(rec[:], den[:])
                        recr = wpool.tile([128, HID], f32, tag="recr")
                        nc.vector.tensor_copy(
                            recr[:].rearrange("p (h c) -> p h c", h=HEADS),
                            rec[:].to_broadcast([128, HEADS, C]))
                        normed = wpool.tile([128, HID], f32, tag="normed")
                        nc.vector.tensor_mul(normed[:], agg[:, 0:HID], recr[:])
                        if layer == 0:
                            nc.vector.tensor_add(normed[:], normed[:],
                                                 b1r_sb[:])
                            relu = wpool.tile([128, HID], f32, tag="relu")
                            nc.vector.tensor_scalar_max(relu[:], normed[:],
                                                        0.0)
                            tp2 = ppool.tile([128, 128], f32, tag="tp")
                            nc.tensor.transpose(tp2[:], relu[:], idf_sb[:])
                            rT = wpool.tile([128, 128], bf16, tag="rT")
                            nc.vector.tensor_copy(rT[:], tp2[:])
                            hp2 = ppool.tile([128, HID + HEADS], f32, tag="hp")
                            nc.tensor.matmul(out=hp2[:, 0:HID], lhsT=rT[:],
                                             rhs=w2t_bf[:], start=True,
                                             stop=True)
                            nc.tensor.matmul(out=hp2[:, HID:HID + HEADS],
                                             lhsT=rT[:], rhs=m2_bf[:],
                                             start=True, stop=True)
                            sl2 = wpool.tile([128, HID], bf16, tag="slice")
                            nc.vector.tensor_copy(sl2[:], hp2[:, 0:HID])
                            nc.vector.tensor_copy(adsb[1][:, b, :],
                                                  hp2[:, HID:HID + HEADS])
                            if b < H0B:
                                nc.sync.dma_start(
                                    gslA[1][b * 128:(b + 1) * 128, :], sl2[:])
                            else:
                                bb = b - H0B
                                nc.sync.dma_start(
                                    gslB[1][bb * 128:(bb + 1) * 128, :],
                                    sl2[:])
                        else:
                            outb = wpool.tile([128, HID], f32, tag="outb")
                            nc.vector.tensor_add(outb[:], normed[:], b2r_sb[:])
                            nc.sync.dma_start(
                                out_t[b * 128:(b + 1) * 128, :], outb[:])
                    if (layer == 0 and K_SPLITCOLL
                            and s == H0B // GSUP - 1):
                        allgather(gslA[1], gf0[1])
                if layer == 0:
                    if not K_SPLITCOLL:
                        allgather(gslA[1], gf0[1])
                    allgather(gslB[1], gf1[1])
    nc.finalize()
    return nc


def kernel(**inputs):
    x = np.asarray(inputs["x"])
    edge_index = np.asarray(inputs["edge_index"])
    emb = np.asarray(inputs["emb"], dtype=np.float32)
    W1 = np.asarray(inputs["W1"], dtype=np.float32)
    W2 = np.asarray(inputs["W2"], dtype=np.float32)
    as1 = np.asarray(inputs["att_src1"], dtype=np.float32)
    ad1 = np.asarray(inputs["att_dst1"], dtype=np.float32)
    as2 = np.asarray(inputs["att_src2"], dtype=np.float32)
    ad2 = np.asarray(inputs["att_dst2"], dtype=np.float32)
    b1 = np.asarray(inputs["b1"], dtype=np.float32)
    b2 = np.asarray(inputs["b2"], dtype=np.float32)

    key = (edge_index.tobytes(), x.tobytes())
    if _cache.get("key") != key:
        meta, cores = _host_layout(x, edge_index)
        nc = _build(bacc.Bacc("TRN2", target_bir_lowering=False, debug=False,
                              enable_asserts=False, num_devices=NCORES,
                              num_swdge_queues=4), meta)
        _cache.update(key=key, nc=nc, cores=cores)
    nc, cores = _cache["nc"], _cache["cores"]

    common = dict(
        emb=emb, w1=W1, w1t=np.ascontiguousarray(W1.T),
        w2=W2, w2t=np.ascontiguousarray(W2.T),
        a1d=_amat_d(ad1),
        a2d=_amat_d(ad2),
        attrep1=np.ascontiguousarray(
            np.broadcast_to(as1.reshape(-1), (128, HID))).astype(BF16),
        attrep2=np.ascontiguousarray(
            np.broadcast_to(as2.reshape(-1), (128, HID))).astype(BF16),
        b1r=np.ascontiguousarray(np.broadcast_to(b1, (128, HID))),
        b2r=np.ascontiguousarray(np.broadcast_to(b2, (128, HID))),
        identf=np.eye(128, dtype=np.float32),
    )
    in_maps = [dict(common, **cores[k]) for k in range(NCORES)]

    res = run_bass_kernel_spmd(nc, in_maps, core_ids=list(range(NCORES)),
                               tmpdir=os.environ.get("BASS_TRACE_DIR"))
    global LAST_EXEC_NS, LAST_TRACE
    LAST_EXEC_NS = res.exec_time_ns
    LAST_TRACE = (res.instructions_and_trace[1]
                  if res.instructions_and_trace else None)
    out = np.concatenate([res.results[k]["out2"][:NSH] for k in range(NCORES)],
                         axis=0)
    return out.astype(np.float32)


LAST_EXEC_NS = None
LAST_TRACE = None


def _amat_d(adst):
    A = np.zeros((HID, HEADS), dtype=np.float32)
    for h in range(HEADS):
        A[h * C:(h + 1) * C, h] = adst[h]
    return A


if __name__ == "__main__":
    import reference
    inputs = {k: np.asarray(v) for k, v in reference.setup_inputs().items()}
    got = kernel(**inputs)
    print("out shape", got.shape, got.dtype)


# revision 21
# speedup vs baseline: 1.8037x; 1.3043x over previous
"""2-layer GAT (GATConv x2, 4 heads, concat) over a 100K-node / 1.7M-edge graph
on 8 Trainium2 NeuronCores.

Destination-sharded graph parallelism:
  - Nodes sharded 12500/core; core k owns destinations [12500k, 12500(k+1)).
  - Per layer each core computes h = x_in @ W.T for its own slice; AllGather
    replicates the full feature table (split in two pieces so the second
    half's collective overlaps the producer loop). Table rows are 256B:
    [h fp8e4m3 x128 | a_src bf16 x4 | pad], so the gathered row carries both
    the message payload and the per-source attention score.
  - The table is addressed by dma_gather int16 indices in 4 chunks
    (32768/32768/32768/2048 rows); one batched dma_gather per (2-block
    super, chunk), round-robined over the 4 SWDGE queues so descriptor
    generation parallelizes across Q7 core pairs.
  - Edge phase per core, per destination block (128 dsts):
      s_e   = a_src[src_e]          free: bf16 columns of the gathered row
      d_e   = a_dst[dst_e]          broadcast by a PE matmul with a
                                    host-precomputed fp8 0/1 matrix SdT[d,e]
      ex_e  = exp(leaky_relu(s_e + d_e))
    One PE matmul per 128-edge group with a host-precomputed fp8 selection
    matrix S01[e,d] accumulates numerators and softmax denominators in PSUM:
      psum[d, 0:128] += sum_e S01[e,d] * ex[e,h] * h[src_e][h,c]
      psum[d,128:132]+= sum_e S01[e,d] * ex[e,h]
    Normalization commutes with the linear aggregation; the per-super
    epilogue (den/recip/mul/bias) runs batched over GSUP blocks, with
    PSUM->SBUF copies and relu on the otherwise-idle ACT engine.
  - Self-loops are ordinary edges; weights replicated; the host does only
    data layout (sharding, sorting, padding, index tables, dtype casts).
"""

import os
import numpy as np
import ml_dtypes

import concourse.bass as bass
import concourse.bacc as bacc
import concourse.tile as tile
import concourse.mybir as mybir
from concourse.bass import IndirectOffsetOnAxis
from concourse.bass_utils import run_bass_kernel_spmd

BF16 = ml_dtypes.bfloat16
FP8 = ml_dtypes.float8_e4m3

N = 100000
EMB_IN = 32
HEADS = 4
C = 32
HID = 128
NEG = 0.2
NCORES = 8
NSH = N // NCORES            # 12500 nodes per shard
NBLK = (NSH + 127) // 128    # 98 dst blocks per shard
NPAD = NBLK * 128            # 12544 padded rows per shard
GSUP = 2                     # blocks per super-batch
NSUP = NBLK // GSUP
PAD_LOC = 300.0              # dst-lane value for padding slots

H0B = 32                     # blocks in table half 0
H0R = H0B * 128              # 4096 rows/core in half 0
H1R = NPAD - H0R             # 8448 rows/core in half 1
CHUNK = 32768

_cache = {}

K_SPLITCOLL = os.environ.get("K_SPLITCOLL", "1") == "1"
K_FP8 = os.environ.get("K_FP8", "1") == "1"


def _table_pos(v):
    """node id -> (chunk, relative row) under the split-table layout."""
    k = v // NSH
    r = v % NSH
    h1 = k * H1R + (r - H0R)
    c = np.where(r < H0R, 0, 1 + h1 // CHUNK)
    rel = np.where(r < H0R, k * H0R + r, h1 % CHUNK)
    return c.astype(np.int64), rel.astype(np.int64)


def _host_layout(x, edge_index):
    """Per-core edge/gather index tables. Pure index manipulation."""
    src = np.concatenate([edge_index[0], np.arange(N, dtype=np.int64)])
    dst = np.concatenate([edge_index[1], np.arange(N, dtype=np.int64)])
    chk, rel = _table_pos(src)

    percore = []
    cnt = np.zeros((NCORES, NBLK, 4), dtype=np.int64)
    for k in range(NCORES):
        lo = k * NSH
        m = (dst >= lo) & (dst < lo + NSH)
        rl, cc = rel[m], chk[m]
        dl = dst[m] - lo
        blk = dl // 128
        order = np.lexsort((cc, blk))
        rl, dl, blk, cc = rl[order], dl[order], blk[order], cc[order]
        for b in range(NBLK):
            bm = blk == b
            cnt[k, b] = np.bincount(cc[bm], minlength=4)
        percore.append((rl, dl, blk, cc))

    # groups per (block, chunk): uniform across cores (SPMD structure)
    gbc = -(-np.max(cnt, axis=0) // 128)          # [NBLK, 4]
    # global group order: super-major, chunk-major, block-minor
    blk_groups = [[] for _ in range(NBLK)]
    grp_blk = []
    sup_specs = []
    gg = 0
    for s in range(NSUP):
        blks = list(range(s * GSUP, (s + 1) * GSUP))
        specs = []
        for c in range(4):
            j0 = gg
            for b in blks:
                for _ in range(int(gbc[b, c])):
                    blk_groups[b].append(gg)
                    grp_blk.append(b)
                    gg += 1
            if gg > j0:
                specs.append((c, j0, gg - j0))
        sup_specs.append(specs)
    gtot = gg

    cores = []
    for k in range(NCORES):
        rl, dl, blk, cc = percore[k]
        idxm = np.zeros((gtot, 128), dtype=np.int16)
        locm = np.full((gtot, 128), PAD_LOC, dtype=np.float32)
        pos = 0
        starts = {}
        for b in range(NBLK):
            for c in range(4):
                n = int(cnt[k, b, c])
                starts[(b, c)] = (pos, n)
                pos += n
        for s in range(NSUP):
            for c, j0, W in sup_specs[s]:
                j = j0
                for b in range(s * GSUP, (s + 1) * GSUP):
                    G = int(gbc[b, c])
                    if G == 0:
                        continue
                    p0, n = starts[(b, c)]
                    vals = np.zeros(G * 128, dtype=np.int16)
                    locs = np.full(G * 128, PAD_LOC, dtype=np.float32)
                    vals[:n] = rl[p0:p0 + n].astype(np.int16)
                    locs[:n] = (dl[p0:p0 + n] - b * 128).astype(np.float32)
                    idxm[j:j + G] = vals.reshape(G, 128)
                    locm[j:j + G] = locs.reshape(G, 128)
                    j += G
        # gidx: idx i of group g -> [16r + i%16, g*8 + i//16]
        gidx16 = idxm.reshape(gtot, 8, 16).transpose(2, 0, 1).reshape(
            16, gtot * 8)
        gidx = np.tile(gidx16, (8, 1))
        loci = locm.astype(np.int64)
        valid = loci < 128
        sdt = np.zeros((128, gtot * 128), dtype=FP8)
        s01 = np.zeros((128, gtot * 128), dtype=FP8)
        ggi, ei = np.nonzero(valid)
        sdt[loci[valid], ggi * 128 + ei] = FP8(1.0)
        s01[ei, ggi * 128 + loci[valid]] = FP8(1.0)
        xsl = np.zeros(NPAD, dtype=np.int32)
        xsl[:NSH] = x[k * NSH:(k + 1) * NSH].astype(np.int32)
        cores.append(dict(gidx=np.ascontiguousarray(gidx),
                          sdt=np.ascontiguousarray(sdt),
                          s01=np.ascontiguousarray(s01),
                          xsh=xsl.reshape(NBLK, 128).T.copy()))
    meta = dict(blk_groups=blk_groups, gtot=gtot, sup_specs=sup_specs,
                grp_blk=grp_blk)
    return meta, cores


def _build(nc, meta):
    dt = mybir.dt
    f32, bf16, i32, i16 = dt.float32, dt.bfloat16, dt.int32, dt.int16
    f8 = dt.float8e4
    gtot = meta["gtot"]
    blk_groups = meta["blk_groups"]
    sup_specs = meta["sup_specs"]
    grp_blk = meta["grp_blk"]
    AF = mybir.ActivationFunctionType
    H8 = 2 * HEADS

    emb_t = nc.dram_tensor("emb", [N, EMB_IN], f32, kind="ExternalInput")
    w1_t = nc.dram_tensor("w1", [HID, EMB_IN], f32, kind="ExternalInput")
    w1t_t = nc.dram_tensor("w1t", [EMB_IN, HID], f32, kind="ExternalInput")
    w2_t = nc.dram_tensor("w2", [HID, HID], f32, kind="ExternalInput")
    w2t_t = nc.dram_tensor("w2t", [HID, HID], f32, kind="ExternalInput")
    # [a_dst | a_src] head-block matrices, [HID, 8]
    a1_t = nc.dram_tensor("a1ds", [HID, H8], f32, kind="ExternalInput")
    a2_t = nc.dram_tensor("a2ds", [HID, H8], f32, kind="ExternalInput")
    at1_t = nc.dram_tensor("attrep1", [128, HID], bf16, kind="ExternalInput")
    at2_t = nc.dram_tensor("attrep2", [128, HID], bf16, kind="ExternalInput")
    b1r_t = nc.dram_tensor("b1r", [128, HID], f32, kind="ExternalInput")
    b2r_t = nc.dram_tensor("b2r", [128, HID], f32, kind="ExternalInput")
    idf_t = nc.dram_tensor("identf", [128, 128], f32, kind="ExternalInput")
    xsh_t = nc.dram_tensor("xsh", [128, NBLK], i32, kind="ExternalInput")
    gidx_t = nc.dram_tensor("gidx", [128, gtot * 8], i16, kind="ExternalInput")
    sdt_t = nc.dram_tensor("sdt", [128, gtot * 128], f8, kind="ExternalInput")
    s01_t = nc.dram_tensor("s01", [128, gtot * 128], f8, kind="ExternalInput")
    out_t = nc.dram_tensor("out2", [NPAD, HID], f32, kind="ExternalOutput")

    gslA = [nc.dram_tensor(f"gslA{l}", [H0R, HID], bf16, kind="Internal")
            for l in (1, 2)]
    gslB = [nc.dram_tensor(f"gslB{l}", [H1R, HID], bf16, kind="Internal")
            for l in (1, 2)]
    gf0 = [nc.dram_tensor(f"gf0_{l}", [H0R * NCORES, HID], bf16,
                          kind="Internal", addr_space="Shared") for l in (1, 2)]
    gf1 = [nc.dram_tensor(f"gf1_{l}", [H1R * NCORES, HID], bf16,
                          kind="Internal", addr_space="Shared") for l in (1, 2)]

    def bmid(ap, w):
        """[128, X] -> [128, w, X] broadcast over a middle dim."""
        return bass.AP(ap.tensor, ap.offset, [list(ap.ap[0]), [0, w],
                                              list(ap.ap[1])])

    def allgather(src, dstt):
        nc.gpsimd.collective_compute(
            "AllGather", mybir.AluOpType.bypass,
            replica_groups=[list(range(NCORES))],
            ins=[src.ap()], outs=[dstt.ap()])

    def acopy(dst, srcv):
        nc.scalar.activation(dst, srcv, AF.Copy)

    with tile.TileContext(nc) as tc:
        with tc.tile_pool(name="const", bufs=1) as cpool, \
             tc.tile_pool(name="work", bufs=2) as wpool, \
             tc.tile_pool(name="psum", bufs=2, space="PSUM") as ppool:

            def cload(t, shape, dtyp):
                s = cpool.tile(shape, dtyp, tag=t.name)
                nc.sync.dma_start(s[:], t[:])
                return s

            w1_sb = cload(w1_t, [HID, EMB_IN], f32)
            w1t_sb = cload(w1t_t, [EMB_IN, HID], f32)
            w2_sb = cload(w2_t, [HID, HID], f32)
            a1_sb = cload(a1_t, [HID, H8], f32)
            a2_sb = cload(a2_t, [HID, H8], f32)
            at1_sb = cload(at1_t, [128, HID], bf16)
            at2_sb = cload(at2_t, [128, HID], bf16)
            at_sb = [at1_sb, at2_sb]
            b1r_sb = cload(b1r_t, [128, HID], f32)
            b2r_sb = cload(b2r_t, [128, HID], f32)
            idf_sb = cload(idf_t, [128, 128], f32)
            xsh_sb = cload(xsh_t, [128, NBLK], i32)
            gidx_sb = cload(gidx_t, [128, gtot * 8], i16)

            w2t_f = cpool.tile([HID, HID], f32, tag="w2tf")
            nc.sync.dma_start(w2t_f[:], w2t_t[:])
            w2t_bf = cpool.tile([HID, HID], bf16, tag="w2tbf")
            nc.vector.tensor_copy(w2t_bf[:], w2t_f[:])

            # M1 = W1^T [A1dst|A1src] [32, 8];  M2 = W2^T [...] [128, 8]
            m1_ps = ppool.tile([EMB_IN, H8], f32, tag="tp")
            nc.tensor.matmul(out=m1_ps[:], lhsT=w1_sb[:], rhs=a1_sb[:],
                             start=True, stop=True)
            m1_sb = cpool.tile([EMB_IN, H8], f32, tag="m1s")
            nc.vector.tensor_copy(m1_sb[:], m1_ps[:])
            m2_ps = ppool.tile([HID, H8], f32, tag="tp")
            nc.tensor.matmul(out=m2_ps[:], lhsT=w2_sb[:], rhs=a2_sb[:],
                             start=True, stop=True)
            m2_bf = cpool.tile([HID, H8], bf16, tag="m2b")
            nc.vector.tensor_copy(m2_bf[:], m2_ps[:])

            # a_dst for own blocks, per layer: [128, NBLK, 4]
            adsb1 = cpool.tile([128, NBLK, HEADS], bf16, tag="adsb1")
            adsb2 = cpool.tile([128, NBLK, HEADS], bf16, tag="adsb2")
            adsb = [adsb1, adsb2]

            # ---- phase A: layer-1 h slice for own nodes ------------------
            for g in range(NBLK):
                embx = wpool.tile([128, EMB_IN], f32, tag="embx")
                nc.gpsimd.indirect_dma_start(
                    out=embx[:], out_offset=None, in_=emb_t[:],
                    in_offset=IndirectOffsetOnAxis(ap=xsh_sb[:, g:g + 1], axis=0))
                tp = ppool.tile([EMB_IN, 128], f32, tag="tp")
                nc.tensor.transpose(tp[:], embx[:], idf_sb[:])
                exT = wpool.tile([EMB_IN, 128], f32, tag="exT")
                acopy(exT[:], tp[:])
                hp = ppool.tile([128, HID + H8], f32, tag="hp")
                nc.tensor.matmul(out=hp[:, 0:HID], lhsT=exT[:], rhs=w1t_sb[:],
                                 start=True, stop=True)
                nc.tensor.matmul(out=hp[:, HID:HID + H8], lhsT=exT[:],
                                 rhs=m1_sb[:], start=True, stop=True)
                sl = wpool.tile([128, HID], bf16, tag="slice")
                if K_FP8:
                    acopy(sl[:, 0:HID // 2].bitcast(f8), hp[:, 0:HID])
                    acopy(sl[:, HID // 2:HID // 2 + HEADS],
                          hp[:, HID + HEADS:HID + H8])
                else:
                    acopy(sl[:], hp[:, 0:HID])
                acopy(adsb[0][:, g, :], hp[:, HID:HID + HEADS])
                if g < H0B:
                    nc.sync.dma_start(gslA[0][g * 128:(g + 1) * 128, :], sl[:])
                else:
                    gb = g - H0B
                    nc.sync.dma_start(gslB[0][gb * 128:(gb + 1) * 128, :],
                                      sl[:])
                if g == H0B - 1 and K_SPLITCOLL:
                    allgather(gslA[0], gf0[0])
            if not K_SPLITCOLL:
                allgather(gslA[0], gf0[0])
            allgather(gslB[0], gf1[0])

            # ---- edge phase ---------------------------------------------
            qload = np.zeros(4, dtype=np.int64)
            for layer in (0, 1):
                cviews = [gf0[layer][:],
                          gf1[layer][0:CHUNK, :],
                          gf1[layer][CHUNK:2 * CHUNK, :],
                          gf1[layer][2 * CHUNK:, :]]
                for s in range(NSUP):
                    gg0 = sup_specs[s][0][1]
                    ggE = sup_specs[s][-1][1] + sup_specs[s][-1][2]
                    Ws = ggE - gg0
                    b0 = s * GSUP
                    gath = wpool.tile([128, Ws, HID], bf16, tag="gath")
                    for cix, j0, W in sup_specs[s]:
                        q = int(np.argmin(qload))
                        qload[q] += W
                        nc.gpsimd.dma_gather(
                            gath[:, j0 - gg0:j0 - gg0 + W, :], cviews[cix],
                            gidx_sb[:, j0 * 8:(j0 + W) * 8],
                            W * 128, W * 128, HID,
                            single_packet=False, queue_num=q)
                    sdt_sb = wpool.tile([128, Ws, 128], f8, tag="sdt")
                    nc.sync.dma_start(
                        sdt_sb[:],
                        sdt_t[:, gg0 * 128:ggE * 128]
                        .rearrange("p (w e) -> p w e", w=Ws))
                    s01_sb = wpool.tile([128, Ws, 128], f8, tag="s01")
                    nc.sync.dma_start(
                        s01_sb[:],
                        s01_t[:, gg0 * 128:ggE * 128]
                        .rearrange("p (w e) -> p w e", w=Ws))
                    # d_e = a_dst[dst_e] via fp8 selection matmul
                    dps = ppool.tile([128, Ws, HEADS], f32, tag="dp")
                    for j in range(Ws):
                        nc.tensor.matmul(out=dps[:, j, :],
                                         lhsT=sdt_sb[:, j, :],
                                         rhs=adsb[layer][:, grp_blk[gg0 + j], :],
                                         start=True, stop=True)
                    # ex = exp(leaky_relu(s + d))
                    z = wpool.tile([128, Ws, HEADS], f32, tag="z")
                    if K_FP8:
                        nc.vector.tensor_add(
                            z[:],
                            gath[:, :, HID // 2:HID // 2 + HEADS], dps[:])
                    else:
                        hm = wpool.tile([128, Ws, HID], bf16, tag="hm")
                        nc.vector.tensor_mul(hm[:], gath[:],
                                             bmid(at_sb[layer][:], Ws))
                        s_sb = wpool.tile([128, Ws, HEADS], f32, tag="s")
                        nc.vector.tensor_reduce(
                            s_sb[:],
                            hm[:].rearrange("p w (h c) -> p w h c", h=HEADS),
                            axis=mybir.AxisListType.X, op=mybir.AluOpType.add)
                        nc.vector.tensor_add(z[:], s_sb[:], dps[:])
                    zm = wpool.tile([128, Ws, HEADS], f32, tag="zm")
                    nc.vector.tensor_scalar_mul(zm[:], z[:], NEG)
                    nc.vector.tensor_max(z[:], z[:], zm[:])
                    ex = wpool.tile([128, Ws, HEADS], bf16, tag="ex")
                    nc.scalar.activation(ex[:], z[:], AF.Exp)
                    # rhs = [h * ex | ex]
                    rhs = wpool.tile([128, Ws, HID + HEADS], bf16, tag="rhs")
                    hsrc = (gath[:, :, 0:HID // 2].bitcast(f8) if K_FP8
                            else gath[:])
                    nc.vector.tensor_mul(
                        rhs[:, :, 0:HID].rearrange("p w (h c) -> p w h c",
                                                   h=HEADS),
                        hsrc.rearrange("p w (h c) -> p w h c", h=HEADS),
                        ex[:].to_broadcast([128, Ws, HEADS, C]))
                    acopy(rhs[:, :, HID:HID + HEADS], ex[:])
                    # aggregate per block into a batched psum tile
                    agg = ppool.tile([128, GSUP, HID + HEADS], f32, tag="agg")
                    for bi in range(GSUP):
                        ggs = blk_groups[b0 + bi]
                        for i, gg in enumerate(ggs):
                            j = gg - gg0
                            nc.tensor.matmul(out=agg[:, bi, :],
                                             lhsT=s01_sb[:, j, :],
                                             rhs=rhs[:, j, :],
                                             start=(i == 0),
                                             stop=(i == len(ggs) - 1))
                    # batched softmax-normalization epilogue
                    den = wpool.tile([128, GSUP, HEADS], f32, tag="den")
                    nc.scalar.activation(den[:], agg[:, :, HID:HID + HEADS],
                                         AF.Copy, bias=1e-16)
                    rec = wpool.tile([128, GSUP, HEADS], f32, tag="rec")
                    nc.vector.reciprocal(rec[:], den[:])
                    recr = wpool.tile([128, GSUP, HID], f32, tag="recr")
                    nc.vector.tensor_copy(
                        recr[:].rearrange("p g (h c) -> p g h c", h=HEADS),
                        rec[:].to_broadcast([128, GSUP, HEADS, C]))
                    normed = wpool.tile([128, GSUP, HID], f32, tag="normed")
                    nc.vector.tensor_mul(normed[:], agg[:, :, 0:HID], recr[:])
                    if layer == 0:
                        nc.vector.tensor_add(normed[:], normed[:],
                                             bmid(b1r_sb[:], GSUP))
                        relu = wpool.tile([128, GSUP, HID], f32, tag="relu")
                        nc.scalar.activation(relu[:], normed[:], AF.Relu)
                        hp2 = ppool.tile([128, GSUP, HID + H8], f32, tag="hp")
                        for bi in range(GSUP):
                            tp2 = ppool.tile([128, 128], f32, tag="tp")
                            nc.tensor.transpose(tp2[:], relu[:, bi, :],
                                                idf_sb[:])
                            rT = wpool.tile([128, 128], bf16, tag="rT")
                            acopy(rT[:], tp2[:])
                            nc.tensor.matmul(out=hp2[:, bi, 0:HID],
                                             lhsT=rT[:], rhs=w2t_bf[:],
                                             start=True, stop=True)
                            nc.tensor.matmul(out=hp2[:, bi, HID:HID + H8],
                                             lhsT=rT[:], rhs=m2_bf[:],
                                             start=True, stop=True)
                        sl2 = wpool.tile([128, GSUP, HID], bf16, tag="slice2")
                        if K_FP8:
                            acopy(sl2[:, :, 0:HID // 2].bitcast(f8),
                                  hp2[:, :, 0:HID])
                            acopy(sl2[:, :, HID // 2:HID // 2 + HEADS],
                                  hp2[:, :, HID + HEADS:HID + H8])
                        else:
                            acopy(sl2[:], hp2[:, :, 0:HID])
                        acopy(adsb[1][:, b0:b0 + GSUP, :],
                              hp2[:, :, HID:HID + HEADS])
                        if b0 < H0B:
                            dst = gslA[1][b0 * 128:(b0 + GSUP) * 128, :]
                        else:
                            bb = b0 - H0B
                            dst = gslB[1][bb * 128:(bb + GSUP) * 128, :]
                        nc.sync.dma_start(
                            dst.rearrange("(g p) c -> p g c", p=128), sl2[:])
                    else:
                        outb = wpool.tile([128, GSUP, HID], f32, tag="outb")
                        nc.vector.tensor_add(outb[:], normed[:],
                                             bmid(b2r_sb[:], GSUP))
                        nc.sync.dma_start(
                            out_t[b0 * 128:(b0 + GSUP) * 128, :]
                            .rearrange("(g p) c -> p g c", p=128), outb[:])
                    if (layer == 0 and K_SPLITCOLL
                            and s == H0B // GSUP - 1):
                        allgather(gslA[1], gf0[1])
                if layer == 0:
                    if not K_SPLITCOLL:
                        allgather(gslA[1], gf0[1])
                    allgather(gslB[1], gf1[1])
    nc.finalize()
    return nc


def kernel(**inputs):
    x = np.asarray(inputs["x"])
    edge_index = np.asarray(inputs["edge_index"])
    emb = np.asarray(inputs["emb"], dtype=np.float32)
    W1 = np.asarray(inputs["W1"], dtype=np.float32)
    W2 = np.asarray(inputs["W2"], dtype=np.float32)
    as1 = np.asarray(inputs["att_src1"], dtype=np.float32)
    ad1 = np.asarray(inputs["att_dst1"], dtype=np.float32)
    as2 = np.asarray(inputs["att_src2"], dtype=np.float32)
    ad2 = np.asarray(inputs["att_dst2"], dtype=np.float32)
    b1 = np.asarray(inputs["b1"], dtype=np.float32)
    b2 = np.asarray(inputs["b2"], dtype=np.float32)

    key = (edge_index.tobytes(), x.tobytes())
    if _cache.get("key") != key:
        meta, cores = _host_layout(x, edge_index)
        nc = _build(bacc.Bacc("TRN2", target_bir_lowering=False, debug=False,
                              enable_asserts=False, num_devices=NCORES,
                              num_swdge_queues=4), meta)
        _cache.update(key=key, nc=nc, cores=cores)
    nc, cores = _cache["nc"], _cache["cores"]

    common = dict(
        emb=emb, w1=W1, w1t=np.ascontiguousarray(W1.T),
        w2=W2, w2t=np.ascontiguousarray(W2.T),
        a1ds=np.concatenate([_amat_d(ad1), _amat_d(as1)], axis=1),
        a2ds=np.concatenate([_amat_d(ad2), _amat_d(as2)], axis=1),
        attrep1=np.ascontiguousarray(
            np.broadcast_to(as1.reshape(-1), (128, HID))).astype(BF16),
        attrep2=np.ascontiguousarray(
            np.broadcast_to(as2.reshape(-1), (128, HID))).astype(BF16),
        b1r=np.ascontiguousarray(np.broadcast_to(b1, (128, HID))),
        b2r=np.ascontiguousarray(np.broadcast_to(b2, (128, HID))),
        identf=np.eye(128, dtype=np.float32),
    )
    in_maps = [dict(common, **cores[k]) for k in range(NCORES)]

    res = run_bass_kernel_spmd(nc, in_maps, core_ids=list(range(NCORES)),
                               tmpdir=os.environ.get("BASS_TRACE_DIR"))
    global LAST_EXEC_NS, LAST_TRACE
    LAST_EXEC_NS = res.exec_time_ns
    LAST_TRACE = (res.instructions_and_trace[1]
                  if res.instructions_and_trace else None)
    out = np.concatenate([res.results[k]["out2"][:NSH] for k in range(NCORES)],
                         axis=0)
    return out.astype(np.float32)


LAST_EXEC_NS = None
LAST_TRACE = None


def _amat_d(adst):
    A = np.zeros((HID, HEADS), dtype=np.float32)
    for h in range(HEADS):
        A[h * C:(h + 1) * C, h] = adst[h]
    return A


if __name__ == "__main__":
    import reference
    inputs = {k: np.asarray(v) for k, v in reference.setup_inputs().items()}
    got = kernel(**inputs)
    print("out shape", got.shape, got.dtype)
